# revision 26
# baseline (speedup 1.0000x reference)
"""Trainium2 Bass kernel v12: full softmax attention, chunked pt WAR fix.

Key discovery (v11 trace): Tile's pool-reuse WAR is whole-tile
granular -- ANY reader of pt(sb) that lingers past the sb boundary
(deferred tree adds, lr DMAs) blocks ALL of sb+1's exp writes into the
reused pt buffer. This was the root of the persistent ~1.6us boundary
stalls in v5/v7 and the 5-12us collapses in v11.

v12: pt and t1 are split into 4 chunk-tiles with separate pool tags
([128, 8*SB] / [128, 4*SB] each, same total SBUF), so sb+1's exps of
chunk c only wait on chunk-c readers of sb -- all of which finish
mid-sb. Tree/lr work may now safely defer across the boundary.
Exp split ACT 21 : DVE 11 (DVE takes the tail tiles 30,31 so the
boundary score slots drain fast); drains alternate ACT/DVE.
"""

import numpy as np
import ml_dtypes
from contextlib import ExitStack

import concourse.bass as bass
import concourse.bacc as bacc
import concourse.mybir as mybir
import concourse.tile as tile
from concourse.bass_utils import run_bass_kernel_spmd

B, S, H, D = 1, 4096, 16, 128
N_CORES = 8
HPC = H // N_CORES
SB = 1024
NSB = S // SB
NKT = S // 128
SCALE = float(1.0 / np.sqrt(D))
BF16 = mybir.dt.bfloat16
FP32 = mybir.dt.float32
I16 = mybir.dt.int16

SCH_SIGMA = 0.05754
SCH_A = float(SCALE * 128.0 / np.log(2.0))
SCH_B = float(128.0 * (127.0 - SCH_SIGMA))
DVE_SET = frozenset((2, 5, 8, 11, 14, 17, 20, 23, 26, 30, 31))

_CACHE = {}


def _build():
    nc = bacc.Bacc("TRN2", target_bir_lowering=False, debug=False)
    qt_d = nc.dram_tensor("qt", [HPC, 128, S], BF16, kind="ExternalInput")
    kt_d = nc.dram_tensor("kt", [HPC, 128, S], BF16, kind="ExternalInput")
    vp_d = nc.dram_tensor("vp", [HPC, 128, S], BF16, kind="ExternalInput")
    o_d = nc.dram_tensor("o", [HPC, NSB, 128, SB], FP32, kind="ExternalOutput")
    lr_d = nc.dram_tensor("lr", [HPC, NSB, 128, 16 * SB], BF16, kind="ExternalOutput")

    with ExitStack() as ctx:
        tc = ctx.enter_context(tile.TileContext(nc))
        qkv = ctx.enter_context(tc.tile_pool(name="qkv", bufs=2))
        ptp = ctx.enter_context(tc.tile_pool(name="ptp", bufs=1))
        trp = ctx.enter_context(tc.tile_pool(name="trp", bufs=1))
        drp = ctx.enter_context(tc.tile_pool(name="drp", bufs=2))

        scp = ctx.enter_context(tc.tile_pool(name="scp", bufs=3, space="PSUM"))
        otp = ctx.enter_context(tc.tile_pool(name="otp", bufs=1, space="PSUM"))

        wsrc = qkv.tile([128, 512], BF16, name="wsrc", tag="wsrc")
        nc.gpsimd.memset(wsrc, 1.0)
        wsc = scp.tile([128, SB], FP32, name="wsc", tag="sc")
        for wi in range(4):
            nc.tensor.matmul(wsc[:, (wi % 2) * 512:(wi % 2) * 512 + 512],
                             wsrc[:, :128], wsrc, start=True, stop=True)

        deferred = []
        pvq = []
        for h in range(HPC):
            qt_s = qkv.tile([128, S], BF16, name=f"qt{h}", tag="qt")
            kt_s = qkv.tile([128, S], BF16, name=f"kt{h}", tag="kt")
            v_s = qkv.tile([128, S], BF16, name=f"v{h}", tag="v")
            if h == 0:
                # need-ordered startup, dual-issued: Sync carries the kt/v
                # streams, Scalar (idle until the first exp ~5us) carries qt.
                # Each dma_start costs ~600ns of issue time on its engine.
                nc.sync.dma_start(qt_s[:, 0:512], qt_d[h][:, 0:512])
                nc.scalar.dma_start(kt_s[:, 0:128], kt_d[h][:, 0:128])
                nc.scalar.dma_start(kt_s[:, 128:512], kt_d[h][:, 128:512])
                nc.sync.dma_start(qt_s[:, 512:1024], qt_d[h][:, 512:1024])
                nc.sync.dma_start(kt_s[:, 512:1024], kt_d[h][:, 512:1024])
                nc.scalar.dma_start(v_s[:, 0:512], vp_d[h][:, 0:512])
                nc.sync.dma_start(kt_s[:, 1024:2048], kt_d[h][:, 1024:2048])
                nc.scalar.dma_start(v_s[:, 512:1024], vp_d[h][:, 512:1024])
                nc.sync.dma_start(kt_s[:, 2048:3072], kt_d[h][:, 2048:3072])
                nc.sync.dma_start(kt_s[:, 3072:4096], kt_d[h][:, 3072:4096])
                for a, b in [(1024, 2048), (2048, 3072), (3072, 4096)]:
                    nc.sync.dma_start(v_s[:, a:b], vp_d[h][:, a:b])
                    nc.sync.dma_start(qt_s[:, a:b], qt_d[h][:, a:b])
            else:
                for a, b in [(0, 1024), (1024, 2048), (2048, 3072), (3072, 4096)]:
                    nc.sync.dma_start(kt_s[:, a:b], kt_d[h][:, a:b])
                    nc.sync.dma_start(qt_s[:, a:b], qt_d[h][:, a:b])
                    nc.sync.dma_start(v_s[:, a:b], vp_d[h][:, a:b])

            for sb in range(NSB):
                q0 = sb * SB
                last = (h == HPC - 1) and (sb == NSB - 1)
                ot = otp.tile([128, SB], FP32, name=f"ot_{h}_{sb}", tag="ot")
                # pt/t1 split into 4 chunk-tiles (separate tags) so the
                # pool-reuse WAR is per-chunk, not whole-sb
                ptc = [ptp.tile([128, 8 * SB], BF16,
                                name=f"pt_{h}_{sb}_{c}", tag=f"pt{c}")
                       for c in range(4)]
                ptc16 = [t.bitcast(I16) for t in ptc]
                t1c = [trp.tile([128, 4 * SB], BF16,
                                name=f"t1_{h}_{sb}_{c}", tag=f"t1{c}")
                       for c in range(4)]

                def pv(j, ot=ot, ptc=ptc, v_s=v_s):
                    vj = v_s[:, j * 128:(j + 1) * 128]
                    pj = ptc[j // 8][:, (j % 8) * SB:(j % 8 + 1) * SB]
                    nc.tensor.matmul(ot[:, :512], vj, pj[:, :512],
                                     start=(j == 0), stop=(j == NKT - 1))
                    nc.tensor.matmul(ot[:, 512:], vj, pj[:, 512:],
                                     start=(j == 0), stop=(j == NKT - 1))

                def l0half(c, hh2, ptc=ptc, t1c=t1c):
                    # one L0 add: 4 pt tiles -> 2 t1 tiles (chunk c half hh2)
                    src = ptc[c][:, hh2 * 4 * SB:(hh2 + 1) * 4 * SB].rearrange(
                        "p (t two q) -> p t two q", two=2, q=SB)
                    dst = t1c[c][:, hh2 * 2 * SB:(hh2 + 1) * 2 * SB].rearrange(
                        "p (t q) -> p t q", q=SB)
                    nc.vector.tensor_add(dst, src[:, :, 0, :], src[:, :, 1, :])

                def lrq(c, h=h, sb=sb, t1c=t1c, last=last):
                    # DMA out lr chunk c ( = t1 chunk c, 4*SB wide )
                    if last:
                        for qq in range(4):
                            c2 = slice(qq * SB, (qq + 1) * SB)
                            dc2 = slice((4 * c + qq) * SB, (4 * c + qq + 1) * SB)
                            nc.sync.dma_start(lr_d[h, sb][:, dc2], t1c[c][:, c2])
                    else:
                        dcs = slice(c * 4 * SB, (c + 1) * 4 * SB)
                        nc.sync.dma_start(lr_d[h, sb][:, dcs], t1c[c])

                for j in range(NKT):
                    sc = scp.tile([128, SB], FP32, name=f"sc_{h}_{sb}_{j}", tag="sc")
                    kj = kt_s[:, j * 128:(j + 1) * 128]
                    nc.tensor.matmul(sc[:, :512], kj, qt_s[:, q0:q0 + 512],
                                     start=True, stop=True)
                    nc.tensor.matmul(sc[:, 512:], kj, qt_s[:, q0 + 512:q0 + SB],
                                     start=True, stop=True)
                    pdst = ptc[j // 8][:, (j % 8) * SB:(j % 8 + 1) * SB]
                    pdst16 = ptc16[j // 8][:, (j % 8) * SB:(j % 8 + 1) * SB]
                    if j in DVE_SET:
                        nc.vector.tensor_scalar(
                            pdst16, sc, SCH_A, SCH_B,
                            mybir.AluOpType.mult, mybir.AluOpType.add)
                    else:
                        nc.scalar.activation(
                            pdst, sc,
                            mybir.ActivationFunctionType.Exp, scale=SCALE)

                    # L0 tree beats on DVE; lr chunk DMA as soon as ready
                    if j == 9:
                        l0half(0, 0)
                    elif j == 10:
                        l0half(0, 1)
                    elif j == 12:
                        lrq(0)
                    elif j == 17:
                        l0half(1, 0)
                    elif j == 19:
                        l0half(1, 1)
                    elif j == 20:
                        lrq(1)
                    elif j == 25:
                        l0half(2, 0)
                    elif j == 27:
                        l0half(2, 1)
                    elif j == 29:
                        l0half(3, 0)
                    elif j == 30:
                        lrq(2)
                    if deferred and j in (1, 3):
                        deferred.pop(0)()

                    def pvstep(j=j, pv=pv, h=h, sb=sb, ot=ot, last=last):
                        pv(j)
                        if j == NKT - 1:
                            # sb epilogue rides with the last PV: drain ot
                            # quarters alternating ACT/DVE (parallel)
                            osb = drp.tile([128, SB], FP32,
                                           name=f"osb_{h}_{sb}", tag="osb")
                            for qq in range(4):
                                cs = slice(qq * SB // 4, (qq + 1) * SB // 4)
                                if qq % 2 == 0:
                                    nc.scalar.copy(osb[:, cs], ot[:, cs])
                                else:
                                    nc.vector.tensor_copy(osb[:, cs], ot[:, cs])
                                nc.sync.dma_start(o_d[h, sb][:, cs], osb[:, cs])
                    pvq.append(pvstep)
                    if len(pvq) > 3:
                        pvq.pop(0)()

                def tailc3b(l0half=l0half):
                    l0half(3, 1)
                def taillr(lrq=lrq):
                    lrq(3)
                if last:
                    while pvq:
                        pvq.pop(0)()
                    tailc3b(); taillr()
                else:
                    deferred.extend([tailc3b, taillr])
        while pvq:
            pvq.pop(0)()
        while deferred:
            deferred.pop(0)()
    nc.compile()
    return nc


def _prep_inputs(q, k, v):
    bf = ml_dtypes.bfloat16
    in_maps = []
    for c in range(N_CORES):
        hs = slice(c * HPC, (c + 1) * HPC)
        qt = np.transpose(q[:, hs, :], (1, 2, 0)).astype(bf)
        kt = np.transpose(k[:, hs, :], (1, 2, 0)).astype(bf)
        vh = np.transpose(v[:, hs, :], (1, 0, 2))
        vp = np.ascontiguousarray(
            vh.reshape(HPC, S // 128, 128, D).transpose(0, 2, 1, 3)
        ).reshape(HPC, 128, S).astype(bf)
        in_maps.append({"qt": qt, "kt": kt, "vp": vp})
    return in_maps


def kernel(q, k, v, ring_size=None, **_unused):
    q = np.asarray(q, dtype=np.float32).reshape(S, H, D)
    k = np.asarray(k, dtype=np.float32).reshape(S, H, D)
    v = np.asarray(v, dtype=np.float32).reshape(S, H, D)

    in_maps = _prep_inputs(q, k, v)
    if "nc" not in _CACHE:
        _CACHE["nc"] = _build()
    res = run_bass_kernel_spmd(_CACHE["nc"], in_maps, list(range(N_CORES))).results

    out = np.empty((B, S, H, D), np.float32)
    for c in range(N_CORES):
        o = np.asarray(res[c]["o"])
        lr = np.asarray(res[c]["lr"]).astype(np.float32)
        for hh in range(HPC):
            l = lr[hh].reshape(NSB, 128, 16, SB).sum(axis=(1, 2))
            on = o[hh] / l[:, None, :]
            out[0, :, c * HPC + hh, :] = on.transpose(0, 2, 1).reshape(S, D)
    return out


# revision 30
# speedup vs baseline: 1.0024x; 1.0024x over previous
"""Trainium2 Bass kernel v12: full softmax attention, chunked pt WAR fix.

Key discovery (v11 trace): Tile's pool-reuse WAR is whole-tile
granular -- ANY reader of pt(sb) that lingers past the sb boundary
(deferred tree adds, lr DMAs) blocks ALL of sb+1's exp writes into the
reused pt buffer. This was the root of the persistent ~1.6us boundary
stalls in v5/v7 and the 5-12us collapses in v11.

v12: pt and t1 are split into 4 chunk-tiles with separate pool tags
([128, 8*SB] / [128, 4*SB] each, same total SBUF), so sb+1's exps of
chunk c only wait on chunk-c readers of sb -- all of which finish
mid-sb. Tree/lr work may now safely defer across the boundary.
Exp split ACT 21 : DVE 11 (DVE takes the tail tiles 30,31 so the
boundary score slots drain fast); drains alternate ACT/DVE.
"""

import numpy as np
import ml_dtypes
from contextlib import ExitStack

import concourse.bass as bass
import concourse.bacc as bacc
import concourse.mybir as mybir
import concourse.tile as tile
from concourse.bass_utils import run_bass_kernel_spmd

B, S, H, D = 1, 4096, 16, 128
N_CORES = 8
HPC = H // N_CORES
SB = 1024
NSB = S // SB
NKT = S // 128
SCALE = float(1.0 / np.sqrt(D))
BF16 = mybir.dt.bfloat16
FP32 = mybir.dt.float32
I16 = mybir.dt.int16

SCH_SIGMA = 0.05754
SCH_A = float(SCALE * 128.0 / np.log(2.0))
SCH_B = float(128.0 * (127.0 - SCH_SIGMA))
DVE_SET = frozenset((2, 5, 8, 11, 14, 17, 20, 23, 26, 30, 31))

_CACHE = {}


def _build():
    nc = bacc.Bacc("TRN2", target_bir_lowering=False, debug=False)
    qt_d = nc.dram_tensor("qt", [HPC, 128, S], BF16, kind="ExternalInput")
    kt_d = nc.dram_tensor("kt", [HPC, 128, S], BF16, kind="ExternalInput")
    vp_d = nc.dram_tensor("vp", [HPC, 128, S], BF16, kind="ExternalInput")
    o_d = nc.dram_tensor("o", [HPC, NSB, 128, SB], FP32, kind="ExternalOutput")
    lr_d = nc.dram_tensor("lr", [HPC, NSB, 128, 16 * SB], BF16, kind="ExternalOutput")

    with ExitStack() as ctx:
        tc = ctx.enter_context(tile.TileContext(nc))
        qkv = ctx.enter_context(tc.tile_pool(name="qkv", bufs=2))
        ptp = ctx.enter_context(tc.tile_pool(name="ptp", bufs=1))
        trp = ctx.enter_context(tc.tile_pool(name="trp", bufs=1))
        drp = ctx.enter_context(tc.tile_pool(name="drp", bufs=2))

        scp = ctx.enter_context(tc.tile_pool(name="scp", bufs=3, space="PSUM"))
        otp = ctx.enter_context(tc.tile_pool(name="otp", bufs=1, space="PSUM"))

        wsrc = qkv.tile([128, 512], BF16, name="wsrc", tag="wsrc")
        nc.vector.memset(wsrc, 1.0)
        wsc = scp.tile([128, SB], FP32, name="wsc", tag="sc")
        for wi in range(10):
            nc.tensor.matmul(wsc[:, (wi % 2) * 512:(wi % 2) * 512 + 512],
                             wsrc[:, :128], wsrc, start=True, stop=True)

        deferred = []
        pvq = []
        pv_target = {3: 4, 4: 5, 5: 5, 6: 4, 7: 3}
        for h in range(HPC):
            qt_s = qkv.tile([128, S], BF16, name=f"qt{h}", tag="qt")
            kt_s = qkv.tile([128, S], BF16, name=f"kt{h}", tag="kt")
            v_s = qkv.tile([128, S], BF16, name=f"v{h}", tag="v")
            if h == 0:
                # need-ordered startup, dual-issued: Sync carries the kt/v
                # streams, Scalar (idle until the first exp ~5us) carries qt.
                # Each dma_start costs ~600ns of issue time on its engine.
                nc.sync.dma_start(kt_s[:, 0:128], kt_d[h][:, 0:128])
                nc.sync.dma_start(qt_s[:, 0:512], qt_d[h][:, 0:512])
                nc.sync.dma_start(kt_s[:, 128:512], kt_d[h][:, 128:512])
                nc.sync.dma_start(qt_s[:, 512:1024], qt_d[h][:, 512:1024])
                nc.sync.dma_start(v_s[:, 0:512], vp_d[h][:, 0:512])
                bounds = [512, 1024, 2048, 3072, 4096]
                for a, b in zip(bounds[:-1], bounds[1:]):
                    nc.sync.dma_start(kt_s[:, a:b], kt_d[h][:, a:b])
                    if a >= 1024:
                        nc.sync.dma_start(qt_s[:, a:b], qt_d[h][:, a:b])
                    nc.sync.dma_start(v_s[:, a:b], vp_d[h][:, a:b])
            else:
                for a, b in [(0, 1024), (1024, 2048), (2048, 3072), (3072, 4096)]:
                    nc.sync.dma_start(kt_s[:, a:b], kt_d[h][:, a:b])
                    nc.sync.dma_start(qt_s[:, a:b], qt_d[h][:, a:b])
                    nc.sync.dma_start(v_s[:, a:b], vp_d[h][:, a:b])

            for sb in range(NSB):
                q0 = sb * SB
                last = (h == HPC - 1) and (sb == NSB - 1)
                ot = otp.tile([128, SB], FP32, name=f"ot_{h}_{sb}", tag="ot")
                # pt/t1 split into 4 chunk-tiles (separate tags) so the
                # pool-reuse WAR is per-chunk, not whole-sb
                ptc = [ptp.tile([128, 8 * SB], BF16,
                                name=f"pt_{h}_{sb}_{c}", tag=f"pt{c}")
                       for c in range(4)]
                ptc16 = [t.bitcast(I16) for t in ptc]
                t1c = [trp.tile([128, 4 * SB], BF16,
                                name=f"t1_{h}_{sb}_{c}", tag=f"t1{c}")
                       for c in range(4)]

                def pv(j, ot=ot, ptc=ptc, v_s=v_s):
                    vj = v_s[:, j * 128:(j + 1) * 128]
                    pj = ptc[j // 8][:, (j % 8) * SB:(j % 8 + 1) * SB]
                    nc.tensor.matmul(ot[:, :512], vj, pj[:, :512],
                                     start=(j == 0), stop=(j == NKT - 1))
                    nc.tensor.matmul(ot[:, 512:], vj, pj[:, 512:],
                                     start=(j == 0), stop=(j == NKT - 1))

                def l0half(c, hh2, ptc=ptc, t1c=t1c):
                    # one L0 add: 4 pt tiles -> 2 t1 tiles (chunk c half hh2)
                    src = ptc[c][:, hh2 * 4 * SB:(hh2 + 1) * 4 * SB].rearrange(
                        "p (t two q) -> p t two q", two=2, q=SB)
                    dst = t1c[c][:, hh2 * 2 * SB:(hh2 + 1) * 2 * SB].rearrange(
                        "p (t q) -> p t q", q=SB)
                    nc.vector.tensor_add(dst, src[:, :, 0, :], src[:, :, 1, :])

                def lrq(c, h=h, sb=sb, t1c=t1c, last=last):
                    # DMA out lr chunk c ( = t1 chunk c, 4*SB wide )
                    if last:
                        for qq in range(4):
                            c2 = slice(qq * SB, (qq + 1) * SB)
                            dc2 = slice((4 * c + qq) * SB, (4 * c + qq + 1) * SB)
                            nc.sync.dma_start(lr_d[h, sb][:, dc2], t1c[c][:, c2])
                    else:
                        dcs = slice(c * 4 * SB, (c + 1) * 4 * SB)
                        nc.sync.dma_start(lr_d[h, sb][:, dcs], t1c[c])

                for j in range(NKT):
                    sc = scp.tile([128, SB], FP32, name=f"sc_{h}_{sb}_{j}", tag="sc")
                    kj = kt_s[:, j * 128:(j + 1) * 128]
                    nc.tensor.matmul(sc[:, :512], kj, qt_s[:, q0:q0 + 512],
                                     start=True, stop=True)
                    nc.tensor.matmul(sc[:, 512:], kj, qt_s[:, q0 + 512:q0 + SB],
                                     start=True, stop=True)
                    pdst = ptc[j // 8][:, (j % 8) * SB:(j % 8 + 1) * SB]
                    pdst16 = ptc16[j // 8][:, (j % 8) * SB:(j % 8 + 1) * SB]
                    if j in DVE_SET:
                        nc.vector.tensor_scalar(
                            pdst16, sc, SCH_A, SCH_B,
                            mybir.AluOpType.mult, mybir.AluOpType.add)
                    else:
                        nc.scalar.activation(
                            pdst, sc,
                            mybir.ActivationFunctionType.Exp, scale=SCALE)

                    # L0 tree beats on DVE; lr chunk DMA as soon as ready
                    if j == 9:
                        l0half(0, 0)
                    elif j == 10:
                        l0half(0, 1)
                    elif j == 12:
                        lrq(0)
                    elif j == 17:
                        l0half(1, 0)
                    elif j == 19:
                        l0half(1, 1)
                    elif j == 20:
                        lrq(1)
                    elif j == 25:
                        l0half(2, 0)
                    elif j == 27:
                        l0half(2, 1)
                    elif j == 29:
                        l0half(3, 0)
                    elif j == 30:
                        lrq(2)
                    if deferred and j in (3, 4):
                        deferred.pop(0)()

                    def pvstep(j=j, pv=pv, h=h, sb=sb, ot=ot, last=last):
                        pv(j)
                        if j == NKT - 1:
                            # sb epilogue rides with the last PV: drain ot
                            # as two wide DVE halves (ACT is the boundary
                            # laggard -- keep it exp-only there)
                            osb = drp.tile([128, SB], FP32,
                                           name=f"osb_{h}_{sb}", tag="osb")
                            for qq in range(2):
                                cs = slice(qq * 512, (qq + 1) * 512)
                                nc.vector.tensor_copy(osb[:, cs], ot[:, cs])
                                nc.sync.dma_start(o_d[h, sb][:, cs], osb[:, cs])
                    pvq.append(pvstep)
                    # delay the new sb's first PV pops so QK0'..4' + the old
                    # sb's PV tail hide the exp31->PV31->drain chain
                    while len(pvq) > pv_target.get(j, 3):
                        pvq.pop(0)()

                def tailc3b(l0half=l0half):
                    l0half(3, 1)
                def taillr(lrq=lrq):
                    lrq(3)
                if last:
                    while pvq:
                        pvq.pop(0)()
                    tailc3b(); taillr()
                else:
                    deferred.extend([tailc3b, taillr])
        while pvq:
            pvq.pop(0)()
        while deferred:
            deferred.pop(0)()
    nc.compile()
    return nc


def _prep_inputs(q, k, v):
    bf = ml_dtypes.bfloat16
    in_maps = []
    for c in range(N_CORES):
        hs = slice(c * HPC, (c + 1) * HPC)
        qt = np.transpose(q[:, hs, :], (1, 2, 0)).astype(bf)
        kt = np.transpose(k[:, hs, :], (1, 2, 0)).astype(bf)
        vh = np.transpose(v[:, hs, :], (1, 0, 2))
        vp = np.ascontiguousarray(
            vh.reshape(HPC, S // 128, 128, D).transpose(0, 2, 1, 3)
        ).reshape(HPC, 128, S).astype(bf)
        in_maps.append({"qt": qt, "kt": kt, "vp": vp})
    return in_maps


def kernel(q, k, v, ring_size=None, **_unused):
    q = np.asarray(q, dtype=np.float32).reshape(S, H, D)
    k = np.asarray(k, dtype=np.float32).reshape(S, H, D)
    v = np.asarray(v, dtype=np.float32).reshape(S, H, D)

    in_maps = _prep_inputs(q, k, v)
    if "nc" not in _CACHE:
        _CACHE["nc"] = _build()
    res = run_bass_kernel_spmd(_CACHE["nc"], in_maps, list(range(N_CORES))).results

    out = np.empty((B, S, H, D), np.float32)
    for c in range(N_CORES):
        o = np.asarray(res[c]["o"])
        lr = np.asarray(res[c]["lr"]).astype(np.float32)
        for hh in range(HPC):
            l = lr[hh].reshape(NSB, 128, 16, SB).sum(axis=(1, 2))
            on = o[hh] / l[:, None, :]
            out[0, :, c * HPC + hh, :] = on.transpose(0, 2, 1).reshape(S, D)
    return out


# revision 31
# speedup vs baseline: 1.0138x; 1.0114x over previous
"""Trainium2 Bass kernel v12: full softmax attention, chunked pt WAR fix.

Key discovery (v11 trace): Tile's pool-reuse WAR is whole-tile
granular -- ANY reader of pt(sb) that lingers past the sb boundary
(deferred tree adds, lr DMAs) blocks ALL of sb+1's exp writes into the
reused pt buffer. This was the root of the persistent ~1.6us boundary
stalls in v5/v7 and the 5-12us collapses in v11.

v12: pt and t1 are split into 4 chunk-tiles with separate pool tags
([128, 8*SB] / [128, 4*SB] each, same total SBUF), so sb+1's exps of
chunk c only wait on chunk-c readers of sb -- all of which finish
mid-sb. Tree/lr work may now safely defer across the boundary.
Exp split ACT 21 : DVE 11 (DVE takes the tail tiles 30,31 so the
boundary score slots drain fast); drains alternate ACT/DVE.
"""

import numpy as np
import ml_dtypes
from contextlib import ExitStack

import concourse.bass as bass
import concourse.bacc as bacc
import concourse.mybir as mybir
import concourse.tile as tile
from concourse.bass_utils import run_bass_kernel_spmd

B, S, H, D = 1, 4096, 16, 128
N_CORES = 8
HPC = H // N_CORES
SB = 1024
NSB = S // SB
NKT = S // 128
SCALE = float(1.0 / np.sqrt(D))
BF16 = mybir.dt.bfloat16
FP32 = mybir.dt.float32
I16 = mybir.dt.int16

SCH_SIGMA = 0.05754
SCH_A = float(SCALE * 128.0 / np.log(2.0))
SCH_B = float(128.0 * (127.0 - SCH_SIGMA))
DVE_SET = frozenset((2, 5, 8, 11, 14, 17, 20, 23, 26, 30, 31))

_CACHE = {}


def _build():
    nc = bacc.Bacc("TRN2", target_bir_lowering=False, debug=False)
    qt_d = nc.dram_tensor("qt", [HPC, 128, S], BF16, kind="ExternalInput")
    kt_d = nc.dram_tensor("kt", [HPC, 128, S], BF16, kind="ExternalInput")
    vp_d = nc.dram_tensor("vp", [HPC, 128, S], BF16, kind="ExternalInput")
    o_d = nc.dram_tensor("o", [HPC, NSB, 128, SB], FP32, kind="ExternalOutput")
    lr_d = nc.dram_tensor("lr", [HPC, NSB, 128, 16 * SB], BF16, kind="ExternalOutput")

    with ExitStack() as ctx:
        tc = ctx.enter_context(tile.TileContext(nc))
        qkv = ctx.enter_context(tc.tile_pool(name="qkv", bufs=2))
        ptp = ctx.enter_context(tc.tile_pool(name="ptp", bufs=1))
        trp = ctx.enter_context(tc.tile_pool(name="trp", bufs=1))
        drp = ctx.enter_context(tc.tile_pool(name="drp", bufs=2))

        scp = ctx.enter_context(tc.tile_pool(name="scp", bufs=3, space="PSUM"))
        otp = ctx.enter_context(tc.tile_pool(name="otp", bufs=1, space="PSUM"))

        wsrc = qkv.tile([128, 512], BF16, name="wsrc", tag="wsrc")
        nc.vector.memset(wsrc, 1.0)
        wsc = scp.tile([128, SB], FP32, name="wsc", tag="sc")
        for wi in range(10):
            nc.tensor.matmul(wsc[:, (wi % 2) * 512:(wi % 2) * 512 + 512],
                             wsrc[:, :128], wsrc, start=True, stop=True)

        deferred = []
        pvq = []
        pv_target = {3: 4, 4: 5, 5: 5, 6: 4, 7: 3}
        for h in range(HPC):
            qt_s = qkv.tile([128, S], BF16, name=f"qt{h}", tag="qt")
            kt_s = qkv.tile([128, S], BF16, name=f"kt{h}", tag="kt")
            v_s = qkv.tile([128, S], BF16, name=f"v{h}", tag="v")
            if h == 0:
                # need-ordered startup, dual-issued: Sync carries the kt/v
                # streams, Scalar (idle until the first exp ~5us) carries qt.
                # Each dma_start costs ~600ns of issue time on its engine.
                nc.sync.dma_start(kt_s[:, 0:128], kt_d[h][:, 0:128])
                nc.sync.dma_start(qt_s[:, 0:512], qt_d[h][:, 0:512])
                nc.sync.dma_start(kt_s[:, 128:512], kt_d[h][:, 128:512])
                nc.sync.dma_start(qt_s[:, 512:1024], qt_d[h][:, 512:1024])
                nc.sync.dma_start(v_s[:, 0:512], vp_d[h][:, 0:512])
                bounds = [512, 1024, 2048, 3072, 4096]
                for a, b in zip(bounds[:-1], bounds[1:]):
                    nc.sync.dma_start(kt_s[:, a:b], kt_d[h][:, a:b])
                    if a >= 1024:
                        nc.sync.dma_start(qt_s[:, a:b], qt_d[h][:, a:b])
                    nc.sync.dma_start(v_s[:, a:b], vp_d[h][:, a:b])
            else:
                for a, b in [(0, 1024), (1024, 2048), (2048, 3072), (3072, 4096)]:
                    nc.sync.dma_start(kt_s[:, a:b], kt_d[h][:, a:b])
                    nc.sync.dma_start(qt_s[:, a:b], qt_d[h][:, a:b])
                    nc.sync.dma_start(v_s[:, a:b], vp_d[h][:, a:b])

            for sb in range(NSB):
                q0 = sb * SB
                last = (h == HPC - 1) and (sb == NSB - 1)
                ot = otp.tile([128, SB], FP32, name=f"ot_{h}_{sb}", tag="ot")
                # pt/t1 split into 4 chunk-tiles (separate tags) so the
                # pool-reuse WAR is per-chunk, not whole-sb
                ptc = [ptp.tile([128, 8 * SB], BF16,
                                name=f"pt_{h}_{sb}_{c}", tag=f"pt{c}")
                       for c in range(4)]
                ptc16 = [t.bitcast(I16) for t in ptc]
                t1c = [trp.tile([128, 4 * SB], BF16,
                                name=f"t1_{h}_{sb}_{c}", tag=f"t1{c}")
                       for c in range(4)]

                def pv(j, ot=ot, ptc=ptc, v_s=v_s):
                    vj = v_s[:, j * 128:(j + 1) * 128]
                    pj = ptc[j // 8][:, (j % 8) * SB:(j % 8 + 1) * SB]
                    nc.tensor.matmul(ot[:, :512], vj, pj[:, :512],
                                     start=(j == 0), stop=(j == NKT - 1))
                    nc.tensor.matmul(ot[:, 512:], vj, pj[:, 512:],
                                     start=(j == 0), stop=(j == NKT - 1))

                def l0half(c, hh2, ptc=ptc, t1c=t1c):
                    # one L0 add: 4 pt tiles -> 2 t1 tiles (chunk c half hh2)
                    src = ptc[c][:, hh2 * 4 * SB:(hh2 + 1) * 4 * SB].rearrange(
                        "p (t two q) -> p t two q", two=2, q=SB)
                    dst = t1c[c][:, hh2 * 2 * SB:(hh2 + 1) * 2 * SB].rearrange(
                        "p (t q) -> p t q", q=SB)
                    nc.vector.tensor_add(dst, src[:, :, 0, :], src[:, :, 1, :])

                def lrq(c, h=h, sb=sb, t1c=t1c, last=last):
                    # DMA out lr chunk c ( = t1 chunk c, 4*SB wide )
                    if last:
                        for qq in range(4):
                            c2 = slice(qq * SB, (qq + 1) * SB)
                            dc2 = slice((4 * c + qq) * SB, (4 * c + qq + 1) * SB)
                            nc.sync.dma_start(lr_d[h, sb][:, dc2], t1c[c][:, c2])
                    else:
                        dcs = slice(c * 4 * SB, (c + 1) * 4 * SB)
                        nc.sync.dma_start(lr_d[h, sb][:, dcs], t1c[c])

                for j in range(NKT):
                    sc = scp.tile([128, SB], FP32, name=f"sc_{h}_{sb}_{j}", tag="sc")
                    kj = kt_s[:, j * 128:(j + 1) * 128]
                    nc.tensor.matmul(sc[:, :512], kj, qt_s[:, q0:q0 + 512],
                                     start=True, stop=True)
                    nc.tensor.matmul(sc[:, 512:], kj, qt_s[:, q0 + 512:q0 + SB],
                                     start=True, stop=True)
                    pdst = ptc[j // 8][:, (j % 8) * SB:(j % 8 + 1) * SB]
                    pdst16 = ptc16[j // 8][:, (j % 8) * SB:(j % 8 + 1) * SB]
                    if j in DVE_SET:
                        nc.vector.tensor_scalar(
                            pdst16, sc, SCH_A, SCH_B,
                            mybir.AluOpType.mult, mybir.AluOpType.add)
                    else:
                        nc.scalar.activation(
                            pdst, sc,
                            mybir.ActivationFunctionType.Exp, scale=SCALE)

                    # L0 tree beats on DVE: one add per 3-beat DVE-exp cycle
                    # (each fits the ~1.34us idle window after a DVE exp;
                    # back-to-back adds would delay the next exp ~1.1us)
                    if j == 9:
                        l0half(0, 0)
                    elif j == 12:
                        l0half(0, 1)
                    elif j == 13:
                        lrq(0)
                    elif j == 18:
                        l0half(1, 0)
                    elif j == 21:
                        l0half(1, 1)
                    elif j == 22:
                        lrq(1)
                    elif j == 24:
                        l0half(2, 0)
                    elif j == 27:
                        l0half(2, 1)
                    elif j == 28:
                        lrq(2)
                    elif j == 29:
                        l0half(3, 0)
                    if deferred and j in (3, 4):
                        deferred.pop(0)()

                    def pvstep(j=j, pv=pv, h=h, sb=sb, ot=ot, last=last):
                        pv(j)
                        if j == NKT - 1:
                            # sb epilogue rides with the last PV: drain ot
                            # as two wide DVE halves (ACT is the boundary
                            # laggard -- keep it exp-only there)
                            osb = drp.tile([128, SB], FP32,
                                           name=f"osb_{h}_{sb}", tag="osb")
                            for qq in range(2):
                                cs = slice(qq * 512, (qq + 1) * 512)
                                nc.vector.tensor_copy(osb[:, cs], ot[:, cs])
                                nc.sync.dma_start(o_d[h, sb][:, cs], osb[:, cs])
                    pvq.append(pvstep)
                    # delay the new sb's first PV pops so QK0'..4' + the old
                    # sb's PV tail hide the exp31->PV31->drain chain
                    while len(pvq) > pv_target.get(j, 3):
                        pvq.pop(0)()

                def tailc3b(l0half=l0half):
                    l0half(3, 1)
                def taillr(lrq=lrq):
                    lrq(3)
                if last:
                    while pvq:
                        pvq.pop(0)()
                    tailc3b(); taillr()
                else:
                    deferred.extend([tailc3b, taillr])
        while pvq:
            pvq.pop(0)()
        while deferred:
            deferred.pop(0)()
    nc.compile()
    return nc


def _prep_inputs(q, k, v):
    bf = ml_dtypes.bfloat16
    in_maps = []
    for c in range(N_CORES):
        hs = slice(c * HPC, (c + 1) * HPC)
        qt = np.transpose(q[:, hs, :], (1, 2, 0)).astype(bf)
        kt = np.transpose(k[:, hs, :], (1, 2, 0)).astype(bf)
        vh = np.transpose(v[:, hs, :], (1, 0, 2))
        vp = np.ascontiguousarray(
            vh.reshape(HPC, S // 128, 128, D).transpose(0, 2, 1, 3)
        ).reshape(HPC, 128, S).astype(bf)
        in_maps.append({"qt": qt, "kt": kt, "vp": vp})
    return in_maps


def kernel(q, k, v, ring_size=None, **_unused):
    q = np.asarray(q, dtype=np.float32).reshape(S, H, D)
    k = np.asarray(k, dtype=np.float32).reshape(S, H, D)
    v = np.asarray(v, dtype=np.float32).reshape(S, H, D)

    in_maps = _prep_inputs(q, k, v)
    if "nc" not in _CACHE:
        _CACHE["nc"] = _build()
    res = run_bass_kernel_spmd(_CACHE["nc"], in_maps, list(range(N_CORES))).results

    out = np.empty((B, S, H, D), np.float32)
    for c in range(N_CORES):
        o = np.asarray(res[c]["o"])
        lr = np.asarray(res[c]["lr"]).astype(np.float32)
        for hh in range(HPC):
            l = lr[hh].reshape(NSB, 128, 16, SB).sum(axis=(1, 2))
            on = o[hh] / l[:, None, :]
            out[0, :, c * HPC + hh, :] = on.transpose(0, 2, 1).reshape(S, D)
    return out


# revision 33
# speedup vs baseline: 1.0205x; 1.0066x over previous
"""Trainium2 Bass kernel v12: full softmax attention, chunked pt WAR fix.

Key discovery (v11 trace): Tile's pool-reuse WAR is whole-tile
granular -- ANY reader of pt(sb) that lingers past the sb boundary
(deferred tree adds, lr DMAs) blocks ALL of sb+1's exp writes into the
reused pt buffer. This was the root of the persistent ~1.6us boundary
stalls in v5/v7 and the 5-12us collapses in v11.

v12: pt and t1 are split into 4 chunk-tiles with separate pool tags
([128, 8*SB] / [128, 4*SB] each, same total SBUF), so sb+1's exps of
chunk c only wait on chunk-c readers of sb -- all of which finish
mid-sb. Tree/lr work may now safely defer across the boundary.
Exp split ACT 21 : DVE 11 (DVE takes the tail tiles 30,31 so the
boundary score slots drain fast); drains alternate ACT/DVE.
"""

import numpy as np
import ml_dtypes
from contextlib import ExitStack

import concourse.bass as bass
import concourse.bacc as bacc
import concourse.mybir as mybir
import concourse.tile as tile
from concourse.bass_utils import run_bass_kernel_spmd

B, S, H, D = 1, 4096, 16, 128
N_CORES = 8
HPC = H // N_CORES
SB = 1024
NSB = S // SB
NKT = S // 128
SCALE = float(1.0 / np.sqrt(D))
BF16 = mybir.dt.bfloat16
FP32 = mybir.dt.float32
I16 = mybir.dt.int16

SCH_SIGMA = 0.05754
SCH_A = float(SCALE * 128.0 / np.log(2.0))
SCH_B = float(128.0 * (127.0 - SCH_SIGMA))
DVE_SET = frozenset((2, 5, 8, 11, 14, 17, 20, 23, 26, 30, 31))

_CACHE = {}


def _build():
    nc = bacc.Bacc("TRN2", target_bir_lowering=False, debug=False)
    qt_d = nc.dram_tensor("qt", [HPC, 128, S], BF16, kind="ExternalInput")
    kt_d = nc.dram_tensor("kt", [HPC, 128, S], BF16, kind="ExternalInput")
    vp_d = nc.dram_tensor("vp", [HPC, 128, S], BF16, kind="ExternalInput")
    o_d = nc.dram_tensor("o", [HPC, NSB, 128, SB], FP32, kind="ExternalOutput")
    lr_d = nc.dram_tensor("lr", [HPC, NSB, 128, 16 * SB], BF16, kind="ExternalOutput")

    with ExitStack() as ctx:
        tc = ctx.enter_context(tile.TileContext(nc))
        qkv = ctx.enter_context(tc.tile_pool(name="qkv", bufs=2))
        ptp = ctx.enter_context(tc.tile_pool(name="ptp", bufs=1))
        trp = ctx.enter_context(tc.tile_pool(name="trp", bufs=1))
        drp = ctx.enter_context(tc.tile_pool(name="drp", bufs=2))

        scp = ctx.enter_context(tc.tile_pool(name="scp", bufs=3, space="PSUM"))
        otp = ctx.enter_context(tc.tile_pool(name="otp", bufs=1, space="PSUM"))

        wsrc = qkv.tile([128, 512], BF16, name="wsrc", tag="wsrc")
        nc.vector.memset(wsrc, 1.0)
        wsc = scp.tile([128, SB], FP32, name="wsc", tag="sc")
        for wi in range(12):
            nc.tensor.matmul(wsc[:, (wi % 2) * 512:(wi % 2) * 512 + 512],
                             wsrc[:, :128], wsrc, start=True, stop=True)

        deferred = []
        pvq = []
        pv_target = {3: 4, 4: 5, 5: 5, 6: 4, 7: 3}
        for h in range(HPC):
            qt_s = qkv.tile([128, S], BF16, name=f"qt{h}", tag="qt")
            kt_s = qkv.tile([128, S], BF16, name=f"kt{h}", tag="kt")
            v_s = qkv.tile([128, S], BF16, name=f"v{h}", tag="v")
            if h == 0:
                # need-ordered startup, dual-issued: Sync carries the kt/v
                # streams, Scalar (idle until the first exp ~5us) carries qt.
                # Each dma_start costs ~600ns of issue time on its engine.
                nc.sync.dma_start(kt_s[:, 0:128], kt_d[h][:, 0:128])
                nc.sync.dma_start(qt_s[:, 0:512], qt_d[h][:, 0:512])
                nc.sync.dma_start(qt_s[:, 512:1024], qt_d[h][:, 512:1024])
                nc.sync.dma_start(kt_s[:, 128:512], kt_d[h][:, 128:512])
                nc.sync.dma_start(v_s[:, 0:512], vp_d[h][:, 0:512])
                nc.sync.dma_start(kt_s[:, 512:1024], kt_d[h][:, 512:1024])
                nc.sync.dma_start(v_s[:, 512:1024], vp_d[h][:, 512:1024])
                for a, b in [(1024, 2048), (2048, 3072), (3072, 4096)]:
                    nc.sync.dma_start(kt_s[:, a:b], kt_d[h][:, a:b])
                    nc.sync.dma_start(v_s[:, a:b], vp_d[h][:, a:b])
                for a, b in [(1024, 2048), (2048, 3072), (3072, 4096)]:
                    nc.sync.dma_start(qt_s[:, a:b], qt_d[h][:, a:b])
            else:
                for a, b in [(0, 1024), (1024, 2048), (2048, 3072), (3072, 4096)]:
                    nc.sync.dma_start(kt_s[:, a:b], kt_d[h][:, a:b])
                    nc.sync.dma_start(qt_s[:, a:b], qt_d[h][:, a:b])
                    nc.sync.dma_start(v_s[:, a:b], vp_d[h][:, a:b])

            for sb in range(NSB):
                q0 = sb * SB
                last = (h == HPC - 1) and (sb == NSB - 1)
                ot = otp.tile([128, SB], FP32, name=f"ot_{h}_{sb}", tag="ot")
                # pt/t1 split into 4 chunk-tiles (separate tags) so the
                # pool-reuse WAR is per-chunk, not whole-sb
                ptc = [ptp.tile([128, 8 * SB], BF16,
                                name=f"pt_{h}_{sb}_{c}", tag=f"pt{c}")
                       for c in range(4)]
                ptc16 = [t.bitcast(I16) for t in ptc]
                t1c = [trp.tile([128, 4 * SB], BF16,
                                name=f"t1_{h}_{sb}_{c}", tag=f"t1{c}")
                       for c in range(4)]

                def pv(j, ot=ot, ptc=ptc, v_s=v_s):
                    vj = v_s[:, j * 128:(j + 1) * 128]
                    pj = ptc[j // 8][:, (j % 8) * SB:(j % 8 + 1) * SB]
                    nc.tensor.matmul(ot[:, :512], vj, pj[:, :512],
                                     start=(j == 0), stop=(j == NKT - 1))
                    nc.tensor.matmul(ot[:, 512:], vj, pj[:, 512:],
                                     start=(j == 0), stop=(j == NKT - 1))

                def l0half(c, hh2, ptc=ptc, t1c=t1c):
                    # one L0 add: 4 pt tiles -> 2 t1 tiles (chunk c half hh2)
                    src = ptc[c][:, hh2 * 4 * SB:(hh2 + 1) * 4 * SB].rearrange(
                        "p (t two q) -> p t two q", two=2, q=SB)
                    dst = t1c[c][:, hh2 * 2 * SB:(hh2 + 1) * 2 * SB].rearrange(
                        "p (t q) -> p t q", q=SB)
                    nc.vector.tensor_add(dst, src[:, :, 0, :], src[:, :, 1, :])

                def lrq(c, h=h, sb=sb, t1c=t1c, last=last):
                    # DMA out lr chunk c ( = t1 chunk c, 4*SB wide )
                    if last:
                        for qq in range(4):
                            c2 = slice(qq * SB, (qq + 1) * SB)
                            dc2 = slice((4 * c + qq) * SB, (4 * c + qq + 1) * SB)
                            nc.sync.dma_start(lr_d[h, sb][:, dc2], t1c[c][:, c2])
                    else:
                        dcs = slice(c * 4 * SB, (c + 1) * 4 * SB)
                        nc.sync.dma_start(lr_d[h, sb][:, dcs], t1c[c])

                for j in range(NKT):
                    sc = scp.tile([128, SB], FP32, name=f"sc_{h}_{sb}_{j}", tag="sc")
                    kj = kt_s[:, j * 128:(j + 1) * 128]
                    nc.tensor.matmul(sc[:, :512], kj, qt_s[:, q0:q0 + 512],
                                     start=True, stop=True)
                    nc.tensor.matmul(sc[:, 512:], kj, qt_s[:, q0 + 512:q0 + SB],
                                     start=True, stop=True)
                    pdst = ptc[j // 8][:, (j % 8) * SB:(j % 8 + 1) * SB]
                    pdst16 = ptc16[j // 8][:, (j % 8) * SB:(j % 8 + 1) * SB]
                    if j in DVE_SET:
                        nc.vector.tensor_scalar(
                            pdst16, sc, SCH_A, SCH_B,
                            mybir.AluOpType.mult, mybir.AluOpType.add)
                    else:
                        nc.scalar.activation(
                            pdst, sc,
                            mybir.ActivationFunctionType.Exp, scale=SCALE)

                    # L0 tree beats on DVE: one add per 3-beat DVE-exp cycle
                    # (each fits the ~1.34us idle window after a DVE exp;
                    # back-to-back adds would delay the next exp ~1.1us)
                    if j == 9:
                        l0half(0, 0)
                    elif j == 12:
                        l0half(0, 1)
                    elif j == 13:
                        lrq(0)
                    elif j == 18:
                        l0half(1, 0)
                    elif j == 21:
                        l0half(1, 1)
                    elif j == 22:
                        lrq(1)
                    elif j == 24:
                        l0half(2, 0)
                    elif j == 27:
                        l0half(2, 1)
                    elif j == 28:
                        lrq(2)
                    elif j == 29:
                        l0half(3, 0)
                    if deferred and j in (3, 4):
                        deferred.pop(0)()

                    def pvstep(j=j, pv=pv, h=h, sb=sb, ot=ot, last=last):
                        pv(j)
                        if j == NKT - 1:
                            # sb epilogue rides with the last PV: drain ot
                            # as two wide DVE halves (ACT is the boundary
                            # laggard -- keep it exp-only there)
                            osb = drp.tile([128, SB], FP32,
                                           name=f"osb_{h}_{sb}", tag="osb")
                            for qq in range(2):
                                cs = slice(qq * 512, (qq + 1) * 512)
                                nc.vector.tensor_copy(osb[:, cs], ot[:, cs])
                                nc.sync.dma_start(o_d[h, sb][:, cs], osb[:, cs])
                    pvq.append(pvstep)
                    # delay the new sb's first PV pops so QK0'..4' + the old
                    # sb's PV tail hide the exp31->PV31->drain chain
                    while len(pvq) > pv_target.get(j, 3):
                        pvq.pop(0)()

                def tailc3b(l0half=l0half):
                    l0half(3, 1)
                def taillr(lrq=lrq):
                    lrq(3)
                if last:
                    while pvq:
                        pvq.pop(0)()
                    tailc3b(); taillr()
                else:
                    deferred.extend([tailc3b, taillr])
        while pvq:
            pvq.pop(0)()
        while deferred:
            deferred.pop(0)()
    nc.compile()
    return nc


def _prep_inputs(q, k, v):
    bf = ml_dtypes.bfloat16
    in_maps = []
    for c in range(N_CORES):
        hs = slice(c * HPC, (c + 1) * HPC)
        qt = np.transpose(q[:, hs, :], (1, 2, 0)).astype(bf)
        kt = np.transpose(k[:, hs, :], (1, 2, 0)).astype(bf)
        vh = np.transpose(v[:, hs, :], (1, 0, 2))
        vp = np.ascontiguousarray(
            vh.reshape(HPC, S // 128, 128, D).transpose(0, 2, 1, 3)
        ).reshape(HPC, 128, S).astype(bf)
        in_maps.append({"qt": qt, "kt": kt, "vp": vp})
    return in_maps


def kernel(q, k, v, ring_size=None, **_unused):
    q = np.asarray(q, dtype=np.float32).reshape(S, H, D)
    k = np.asarray(k, dtype=np.float32).reshape(S, H, D)
    v = np.asarray(v, dtype=np.float32).reshape(S, H, D)

    in_maps = _prep_inputs(q, k, v)
    if "nc" not in _CACHE:
        _CACHE["nc"] = _build()
    res = run_bass_kernel_spmd(_CACHE["nc"], in_maps, list(range(N_CORES))).results

    out = np.empty((B, S, H, D), np.float32)
    for c in range(N_CORES):
        o = np.asarray(res[c]["o"])
        lr = np.asarray(res[c]["lr"]).astype(np.float32)
        for hh in range(HPC):
            l = lr[hh].reshape(NSB, 128, 16, SB).sum(axis=(1, 2))
            on = o[hh] / l[:, None, :]
            out[0, :, c * HPC + hh, :] = on.transpose(0, 2, 1).reshape(S, D)
    return out


# revision 34
# speedup vs baseline: 1.0217x; 1.0012x over previous
"""Trainium2 Bass kernel v17: full softmax attention, 2 heads/core.

Design (trace-driven, from the 269us v5 baseline to ~253us):
  - per-core work: 2 heads x (QK^T, exp, P@V, denominator) over the
    full 4096 sequence; no inter-core communication (head-parallel).
  - PE: 128 MMs of N=512 per 1024-q block; both QK and PV reuse each
    128-wide weight for 2 MMs. Floor is ~216ns/MM (streaming-bound).
  - exp split ACT 21 : DVE 11 tiles (DVE uses the int16-bitcast
    Schraudolph exp2 trick); DVE also owns the 8 L0 tree adds, spread
    one per 3-beat DVE-exp cycle so each fits the idle window.
  - softmax denominator: L0 (pairwise) on-chip only; 16 partial tiles
    per (h,sb) go to HBM ([128,16*SB] bf16) and the host finishes.
  - pt/t1 are split into 4 chunk-tiles with separate pool tags: Tile's
    pool-reuse WAR is whole-tile granular, so a lingering cross-sb
    reader of a monolithic pt would block ALL of the next sb's exps
    (this was the root of the v5/v7 boundary stalls).
  - boundary: tail tiles 30,31 exp on DVE (fast slot drain), ot drains
    are two wide DVE copies, and the next sb's first PV pops late
    (pv_target) so QK0'..4' hide the exp31->PV31->drain chain.
  - startup: need-ordered DMAs with qt[512:1024] promoted (needed at
    beat 0) + 12 warmup MMs sized to bridge to data-ready without a
    >3.4us PE idle (which would re-throttle HAM to 1.2GHz).
"""

import numpy as np
import ml_dtypes
from contextlib import ExitStack

import concourse.bass as bass
import concourse.bacc as bacc
import concourse.mybir as mybir
import concourse.tile as tile
from concourse.bass_utils import run_bass_kernel_spmd

B, S, H, D = 1, 4096, 16, 128
N_CORES = 8
HPC = H // N_CORES
SB = 1024
NSB = S // SB
NKT = S // 128
SCALE = float(1.0 / np.sqrt(D))
BF16 = mybir.dt.bfloat16
FP32 = mybir.dt.float32
I16 = mybir.dt.int16

SCH_SIGMA = 0.05754
SCH_A = float(SCALE * 128.0 / np.log(2.0))
SCH_B = float(128.0 * (127.0 - SCH_SIGMA))
DVE_SET = frozenset((2, 5, 8, 11, 14, 17, 20, 23, 26, 30, 31))

_CACHE = {}


def _build():
    nc = bacc.Bacc("TRN2", target_bir_lowering=False, debug=False)
    qt_d = nc.dram_tensor("qt", [HPC, 128, S], BF16, kind="ExternalInput")
    kt_d = nc.dram_tensor("kt", [HPC, 128, S], BF16, kind="ExternalInput")
    vp_d = nc.dram_tensor("vp", [HPC, 128, S], BF16, kind="ExternalInput")
    o_d = nc.dram_tensor("o", [HPC, NSB, 128, SB], FP32, kind="ExternalOutput")
    lr_d = nc.dram_tensor("lr", [HPC, NSB, 128, 16 * SB], BF16, kind="ExternalOutput")

    with ExitStack() as ctx:
        tc = ctx.enter_context(tile.TileContext(nc))
        qkv = ctx.enter_context(tc.tile_pool(name="qkv", bufs=2))
        ptp = ctx.enter_context(tc.tile_pool(name="ptp", bufs=1))
        trp = ctx.enter_context(tc.tile_pool(name="trp", bufs=1))
        drp = ctx.enter_context(tc.tile_pool(name="drp", bufs=2))

        scp = ctx.enter_context(tc.tile_pool(name="scp", bufs=3, space="PSUM"))
        otp = ctx.enter_context(tc.tile_pool(name="otp", bufs=1, space="PSUM"))

        wsrc = qkv.tile([128, 512], BF16, name="wsrc", tag="wsrc")
        nc.vector.memset(wsrc, 1.0)
        wsc = scp.tile([128, SB], FP32, name="wsc", tag="sc")
        for wi in range(12):
            nc.tensor.matmul(wsc[:, (wi % 2) * 512:(wi % 2) * 512 + 512],
                             wsrc[:, :128], wsrc, start=True, stop=True)

        deferred = []
        pvq = []
        pv_target = {3: 4, 4: 5, 5: 5, 6: 4, 7: 3}
        for h in range(HPC):
            qt_s = qkv.tile([128, S], BF16, name=f"qt{h}", tag="qt")
            kt_s = qkv.tile([128, S], BF16, name=f"kt{h}", tag="kt")
            v_s = qkv.tile([128, S], BF16, name=f"v{h}", tag="v")
            if h == 0:
                # need-ordered startup, dual-issued: Sync carries the kt/v
                # streams, Scalar (idle until the first exp ~5us) carries qt.
                # Each dma_start costs ~600ns of issue time on its engine.
                nc.sync.dma_start(kt_s[:, 0:128], kt_d[h][:, 0:128])
                nc.sync.dma_start(qt_s[:, 0:512], qt_d[h][:, 0:512])
                nc.sync.dma_start(qt_s[:, 512:1024], qt_d[h][:, 512:1024])
                nc.sync.dma_start(kt_s[:, 128:512], kt_d[h][:, 128:512])
                nc.sync.dma_start(v_s[:, 0:512], vp_d[h][:, 0:512])
                nc.sync.dma_start(kt_s[:, 512:1024], kt_d[h][:, 512:1024])
                nc.sync.dma_start(v_s[:, 512:1024], vp_d[h][:, 512:1024])
                for a, b in [(1024, 2048), (2048, 3072), (3072, 4096)]:
                    nc.sync.dma_start(kt_s[:, a:b], kt_d[h][:, a:b])
                    nc.sync.dma_start(v_s[:, a:b], vp_d[h][:, a:b])
                for a, b in [(1024, 2048), (2048, 3072), (3072, 4096)]:
                    nc.sync.dma_start(qt_s[:, a:b], qt_d[h][:, a:b])
            else:
                for a, b in [(0, 1024), (1024, 2048), (2048, 3072), (3072, 4096)]:
                    nc.sync.dma_start(kt_s[:, a:b], kt_d[h][:, a:b])
                    nc.sync.dma_start(qt_s[:, a:b], qt_d[h][:, a:b])
                    nc.sync.dma_start(v_s[:, a:b], vp_d[h][:, a:b])

            for sb in range(NSB):
                q0 = sb * SB
                last = (h == HPC - 1) and (sb == NSB - 1)
                ot = otp.tile([128, SB], FP32, name=f"ot_{h}_{sb}", tag="ot")
                # pt/t1 split into 4 chunk-tiles (separate tags) so the
                # pool-reuse WAR is per-chunk, not whole-sb
                ptc = [ptp.tile([128, 8 * SB], BF16,
                                name=f"pt_{h}_{sb}_{c}", tag=f"pt{c}")
                       for c in range(4)]
                ptc16 = [t.bitcast(I16) for t in ptc]
                t1c = [trp.tile([128, 4 * SB], BF16,
                                name=f"t1_{h}_{sb}_{c}", tag=f"t1{c}")
                       for c in range(4)]

                def pv(j, ot=ot, ptc=ptc, v_s=v_s):
                    vj = v_s[:, j * 128:(j + 1) * 128]
                    pj = ptc[j // 8][:, (j % 8) * SB:(j % 8 + 1) * SB]
                    nc.tensor.matmul(ot[:, :512], vj, pj[:, :512],
                                     start=(j == 0), stop=(j == NKT - 1))
                    nc.tensor.matmul(ot[:, 512:], vj, pj[:, 512:],
                                     start=(j == 0), stop=(j == NKT - 1))

                def l0half(c, hh2, ptc=ptc, t1c=t1c):
                    # one L0 add: 4 pt tiles -> 2 t1 tiles (chunk c half hh2)
                    src = ptc[c][:, hh2 * 4 * SB:(hh2 + 1) * 4 * SB].rearrange(
                        "p (t two q) -> p t two q", two=2, q=SB)
                    dst = t1c[c][:, hh2 * 2 * SB:(hh2 + 1) * 2 * SB].rearrange(
                        "p (t q) -> p t q", q=SB)
                    nc.vector.tensor_add(dst, src[:, :, 0, :], src[:, :, 1, :])

                def lrq(c, h=h, sb=sb, t1c=t1c, last=last):
                    # DMA out lr chunk c ( = t1 chunk c, 4*SB wide )
                    if last:
                        for qq in range(4):
                            c2 = slice(qq * SB, (qq + 1) * SB)
                            dc2 = slice((4 * c + qq) * SB, (4 * c + qq + 1) * SB)
                            nc.sync.dma_start(lr_d[h, sb][:, dc2], t1c[c][:, c2])
                    else:
                        dcs = slice(c * 4 * SB, (c + 1) * 4 * SB)
                        nc.sync.dma_start(lr_d[h, sb][:, dcs], t1c[c])

                for j in range(NKT):
                    sc = scp.tile([128, SB], FP32, name=f"sc_{h}_{sb}_{j}", tag="sc")
                    kj = kt_s[:, j * 128:(j + 1) * 128]
                    nc.tensor.matmul(sc[:, :512], kj, qt_s[:, q0:q0 + 512],
                                     start=True, stop=True)
                    nc.tensor.matmul(sc[:, 512:], kj, qt_s[:, q0 + 512:q0 + SB],
                                     start=True, stop=True)
                    pdst = ptc[j // 8][:, (j % 8) * SB:(j % 8 + 1) * SB]
                    pdst16 = ptc16[j // 8][:, (j % 8) * SB:(j % 8 + 1) * SB]
                    if j in DVE_SET:
                        nc.vector.tensor_scalar(
                            pdst16, sc, SCH_A, SCH_B,
                            mybir.AluOpType.mult, mybir.AluOpType.add)
                    else:
                        nc.scalar.activation(
                            pdst, sc,
                            mybir.ActivationFunctionType.Exp, scale=SCALE)

                    # L0 tree beats on DVE: one add per 3-beat DVE-exp cycle
                    # (each fits the ~1.34us idle window after a DVE exp;
                    # back-to-back adds would delay the next exp ~1.1us)
                    if j == 9:
                        l0half(0, 0)
                    elif j == 12:
                        l0half(0, 1)
                    elif j == 13:
                        lrq(0)
                    elif j == 18:
                        l0half(1, 0)
                    elif j == 21:
                        l0half(1, 1)
                    elif j == 22:
                        lrq(1)
                    elif j == 24:
                        l0half(2, 0)
                    elif j == 27:
                        l0half(2, 1)
                    elif j == 28:
                        lrq(2)
                    elif j == 29:
                        l0half(3, 0)
                    if deferred and j in (3, 4):
                        deferred.pop(0)()

                    def pvstep(j=j, pv=pv, h=h, sb=sb, ot=ot, last=last):
                        pv(j)
                        if j == NKT - 1:
                            # sb epilogue rides with the last PV: drain ot
                            # as two wide DVE halves (ACT is the boundary
                            # laggard -- keep it exp-only there)
                            osb = drp.tile([128, SB], FP32,
                                           name=f"osb_{h}_{sb}", tag="osb")
                            for qq in range(2):
                                cs = slice(qq * 512, (qq + 1) * 512)
                                nc.vector.tensor_copy(osb[:, cs], ot[:, cs])
                                nc.sync.dma_start(o_d[h, sb][:, cs], osb[:, cs])
                    pvq.append(pvstep)
                    # delay the new sb's first PV pops so QK0'..4' + the old
                    # sb's PV tail hide the exp31->PV31->drain chain
                    while len(pvq) > pv_target.get(j, 3):
                        pvq.pop(0)()

                def tailc3b(l0half=l0half):
                    l0half(3, 1)
                def taillr(lrq=lrq):
                    lrq(3)
                if last:
                    while pvq:
                        pvq.pop(0)()
                    tailc3b(); taillr()
                else:
                    deferred.extend([tailc3b, taillr])
        while pvq:
            pvq.pop(0)()
        while deferred:
            deferred.pop(0)()
    nc.compile()
    return nc


def _prep_inputs(q, k, v):
    bf = ml_dtypes.bfloat16
    in_maps = []
    for c in range(N_CORES):
        hs = slice(c * HPC, (c + 1) * HPC)
        qt = np.transpose(q[:, hs, :], (1, 2, 0)).astype(bf)
        kt = np.transpose(k[:, hs, :], (1, 2, 0)).astype(bf)
        vh = np.transpose(v[:, hs, :], (1, 0, 2))
        vp = np.ascontiguousarray(
            vh.reshape(HPC, S // 128, 128, D).transpose(0, 2, 1, 3)
        ).reshape(HPC, 128, S).astype(bf)
        in_maps.append({"qt": qt, "kt": kt, "vp": vp})
    return in_maps


def kernel(q, k, v, ring_size=None, **_unused):
    q = np.asarray(q, dtype=np.float32).reshape(S, H, D)
    k = np.asarray(k, dtype=np.float32).reshape(S, H, D)
    v = np.asarray(v, dtype=np.float32).reshape(S, H, D)

    in_maps = _prep_inputs(q, k, v)
    if "nc" not in _CACHE:
        _CACHE["nc"] = _build()
    res = run_bass_kernel_spmd(_CACHE["nc"], in_maps, list(range(N_CORES))).results

    out = np.empty((B, S, H, D), np.float32)
    for c in range(N_CORES):
        o = np.asarray(res[c]["o"])
        lr = np.asarray(res[c]["lr"]).astype(np.float32)
        for hh in range(HPC):
            l = lr[hh].reshape(NSB, 128, 16, SB).sum(axis=(1, 2))
            on = o[hh] / l[:, None, :]
            out[0, :, c * HPC + hh, :] = on.transpose(0, 2, 1).reshape(S, D)
    return out
